# revision 2
# baseline (speedup 1.0000x reference)
"""Self-attention (SAGAN-style, spectral-normalized 1x1 convs) on 8 TRN2 cores.

Contract: kernel(**inputs) takes the FULL unsharded inputs
(x [8,512,64,64], weights, power-iteration u vectors, gamma) and returns
the FULL output [8,512,64,64] (float32).

Sharding: data-parallel over batch B=8 -> one batch element per core.
Each core runs the complete attention block for its element; no
collectives are needed.

Per-core math (C=512, HW=4096, M=HW/4=1024):
    theta = sn(w_theta) @ x          [64, 4096]
    phi   = maxpool2(sn(w_phi) @ x)  [64, 1024]
    g     = maxpool2(sn(w_g)   @ x)  [256, 1024]
    sT[m,n] = sum_c phi[c,m] theta[c,n]
    beta  = softmax over m  (exp without max-subtraction: logits span
            ~+-51, safe in fp32/bf16; normalization is applied to o)
    o     = (g @ exp(sT)) * (1/sum)  [256, 4096]
    out   = gamma * (sn(w_o) @ o) + x

Precision plan (measured 5.2e-3 output rel err vs the 2e-2 gate):
 - logits path fp16 (x16, w_theta|w_phi fused, theta, phi)
 - g-projection in fp8 e4m3 with DoubleRow matmuls (2x PE rate);
   wg is pre-scaled by 8 on the host so its entries are e4m3-normal,
   compensated exactly by using 8.0 (not 1.0) in the ones-matmul that
   produces the softmax denominators
 - value path bf16 (E=exp(sT), g, gT, wo with gamma folded), fp32 PSUM
 - residual + output in fp16 (out dram tensor is fp16, host upcasts)

Engine budget per 512-column attention block (the steady-state loop):
 - PE    ~6.3us: 4 packed sT pairs, 1 ones-matmul (sum over m partitions
   of the DVE-pre-reduced exp), 16 o-matmuls, 8 out-proj matmuls
 - ACT   ~5.6us: 4 exp activations + 2 PSUM->SBUF copies (out staging)
 - DVE   ~5.0us: exp partial-reduce tree (2 levels), fast reciprocal,
   o-normalize, 2 fused residual adds
 - GPSIMD ~3.3us: last reduce level + 2 residual adds (SBUF-only engine)
Host inputs arrive pre-converted (x fp16 + x fp8 + weights), so there is
no on-device dtype conversion of x at all; softmax sums use a 3-level
reduce tree + ONE ones-matmul instead of 8 matmuls per block; the exact
`reciprocal` (6.4 cyc/elem) is replaced by reciprocal_approx_fast.

PE->PE self-waits are stripped (PE->PSUM write port is FIFO) and bacc's
generate_event_semaphores legalizes the 1-wait ISA limit.

The spectral-norm power-iteration only involves [1,64]x[64,512]
matvecs, so it runs on the host in float32; gamma is folded into w_o.
"""

import numpy as np

B, C, H, W = 8, 512, 64, 64
HW = H * W            # 4096
M = HW // 4           # 1024 (pooled spatial)
C8 = C // 8           # 64
C2 = C // 2           # 256
P = 128               # SBUF partitions
KC = C // P           # 4 k-chunks for C-contraction
FB = 512              # free-dim block
NB = HW // FB         # 8 n-blocks
MC = M // P           # 8 m-chunks
WG_SCALE = 8.0        # host-side wg scale (e4m3 normal range); the
                      # ones-matmul uses this value to compensate exactly
EPS = 1e-12

_CACHE = {}


def _sn(w, u):
    """Host-side spectral norm (eval-mode power iteration), float32."""
    w = np.asarray(w, np.float32)
    u = np.asarray(u, np.float32)
    v = u @ w
    v = v / max(np.float32(np.linalg.norm(v)), np.float32(EPS))
    u2 = v @ w.T
    u2 = u2 / max(np.float32(np.linalg.norm(u2)), np.float32(EPS))
    sv = np.float32((v @ w.T @ u2.T)[0, 0])
    return w / sv


def _strip_pe_self_waits(nc):
    """Remove S[PE]-waits from PE matmuls: PE->PE deps are ordered by the
    engine queue + FIFO PSUM write port, and matmuls only have one ISA
    wait slot."""
    import concourse.mybir as mybir

    for f in nc.m.functions:
        for blk in f.blocks:
            for inst in blk.instructions:
                if not isinstance(inst, mybir.InstMatmult):
                    continue
                si = inst.sync_info
                kept = [w for w in si.on_wait
                        if not (w.ant_name or "").startswith("PE_")]
                if len(kept) != len(si.on_wait):
                    si.on_wait = kept
                    inst.sync_info = si


def _build_nc():
    import concourse.bass as bass
    import concourse.mybir as mybir
    import concourse.tile as tile
    from concourse import bacc
    from concourse.masks import make_identity

    fp32 = mybir.dt.float32
    fp16 = mybir.dt.float16
    bf16 = mybir.dt.bfloat16
    fp8 = mybir.dt.float8e4
    DR = mybir.MatmulPerfMode.DoubleRow
    Exp = mybir.ActivationFunctionType.Exp
    mult = mybir.AluOpType.mult
    add = mybir.AluOpType.add
    mx = mybir.AluOpType.max

    nc = bacc.Bacc()
    x16_d = nc.dram_tensor("x16", [C, HW], fp16, kind="ExternalInput").ap()
    x8_d = nc.dram_tensor("x8", [C, HW], fp8, kind="ExternalInput").ap()
    wtp_d = nc.dram_tensor("wtp", [C, P], fp16, kind="ExternalInput").ap()
    wg_d = nc.dram_tensor("wg", [C, C2], fp8, kind="ExternalInput").ap()
    wo_d = nc.dram_tensor("wo", [C2, C], bf16, kind="ExternalInput").ap()
    out_d = nc.dram_tensor("out", [C, HW], fp16, kind="ExternalOutput").ap()

    x16_r = x16_d.rearrange("(kc p) n -> p kc n", p=P)
    x8_r = x8_d.rearrange("(kc p) n -> p kc n", p=P)
    out_r = out_d.rearrange("(ig p) n -> p ig n", p=P)

    with tile.TileContext(nc) as tc:
        with tc.tile_pool(name="sb", bufs=1) as sb:
            # ---- persistent tiles ----
            x16 = sb.tile([P, KC, HW], fp16)
            x8 = sb.tile([P, KC, HW], fp8)
            wtp = sb.tile([P, KC, P], fp16)
            wg8 = sb.tile([P, KC, C2], fp8)
            wo2 = sb.tile([P, 2, C], bf16)
            theta_sb = sb.tile([P, HW], fp16)             # rows 64:128 duplicate
            phi2 = sb.tile([P, NB, 4, 32], fp16)          # rows 64:128 duplicate
            g2 = sb.tile([P, 2, M], bf16)                 # pooled, cg-major, = 8*g
            gT_sb = sb.tile([P, MC, C2], bf16)            # [m-part, mc, c]
            identity = sb.tile([P, P], bf16)
            ones_mat = sb.tile([P, P], bf16)              # value WG_SCALE

            phi_flat = phi2.rearrange("p a b c -> p (a b c)")
            g4 = g2.rearrange("p cg (fb h2 w2) -> p cg fb h2 w2", h2=4, w2=32)

            # ---- constants + HAM warmup ----
            # ones holds WG_SCALE so the softmax denominators come out as
            # WG_SCALE*sum(E), cancelling the host-side wg scaling in the
            # o-normalize.
            nc.vector.memset(ones_mat, WG_SCALE)
            with tc.tile_pool(name="psW", bufs=1, space="PSUM") as psW:
                # a few junk matmuls keep the PE busy during the DMA head so
                # the HAM clock-gate opens before the real matmul stream
                wps = psW.tile([P, P], fp32, tag="warm")
                for _ in range(6):
                    nc.tensor.matmul(wps, lhsT=ones_mat, rhs=ones_mat,
                                     start=True, stop=True)
            ident_raw = sb.tile([P, P], fp32)
            make_identity(nc, ident_raw)
            nc.scalar.copy(identity, ident_raw)

            # ---- input DMAs (pre-converted on host; two queues) ----
            nc.sync.dma_start(wtp, wtp_d.rearrange("(kc p) i -> p kc i", p=P))
            nc.gpsimd.dma_start(wg8, wg_d.rearrange("(kc p) i -> p kc i", p=P))
            nc.sync.dma_start(wo2, wo_d.rearrange("(cg p) i -> p cg i", p=P))
            for q in range(NB):
                sl = slice(q * FB, (q + 1) * FB)
                nc.sync.dma_start(x16[:, :, sl], x16_r[:, :, sl])
                nc.gpsimd.dma_start(x8[:, :, sl], x8_r[:, :, sl])

            # ---------- projections ----------
            with (
                tc.tile_pool(name="psA", bufs=3, space="PSUM") as psA,
                tc.tile_pool(name="psT", bufs=2, space="PSUM") as psT,
            ):
                # fused theta+phi projection: lhsT = [wt | wp] gives
                # theta on out-partitions 0:64, phi on 64:128; both results
                # are duplicated onto partitions 64:128 for sT row-packing.
                for fb2 in range(NB // 2):
                    ps = psA.tile([P, 2, FB], fp32, tag="proj", name="ps")
                    for half in range(2):
                        for kc in range(KC):
                            nc.tensor.matmul(
                                ps[:, half, :],
                                lhsT=wtp[:, kc, :],
                                rhs=x16[:, kc, (2 * fb2 + half) * FB:(2 * fb2 + half + 1) * FB],
                                start=(kc == 0), stop=(kc == KC - 1),
                            )
                    th = ps[:C8].rearrange("p a b -> p (a b)")
                    thsl = slice(2 * fb2 * FB, (2 * fb2 + 2) * FB)
                    nc.scalar.copy(theta_sb[:C8, thsl], th)
                    nc.vector.tensor_copy(theta_sb[C8:, thsl], theta_sb[:C8, thsl])
                    v = ps[C8:].rearrange("p fb (h2 hr w2 wr) -> p fb h2 hr w2 wr",
                                          hr=2, w2=32, wr=2)
                    dst = phi2[:C8, 2 * fb2:2 * fb2 + 2]   # [64, 2, 4, 32]
                    nc.vector.tensor_copy(dst, v[:, :, :, 0, :, 0])
                    nc.vector.tensor_tensor(dst, dst, v[:, :, :, 0, :, 1], mx)
                    nc.vector.tensor_tensor(dst, dst, v[:, :, :, 1, :, 0], mx)
                    nc.vector.tensor_tensor(dst, dst, v[:, :, :, 1, :, 1], mx)
                    nc.vector.tensor_copy(phi2[C8:, 2 * fb2:2 * fb2 + 2],
                                          phi2[:C8, 2 * fb2:2 * fb2 + 2])

                    # g projection in fp8 DoubleRow (k=256 per matmul) +
                    # maxpool on the same x8 columns
                    for cg in range(2):
                        ps = psA.tile([P, 2, FB], fp32, tag="proj", name="psg")
                        for half in range(2):
                            hsl = slice((2 * fb2 + half) * FB,
                                        (2 * fb2 + half + 1) * FB)
                            for qq in range(2):
                                nc.tensor.matmul(
                                    ps[:, half, :],
                                    lhsT=wg8[:, 2 * qq:2 * qq + 2, cg * P:(cg + 1) * P],
                                    rhs=x8[:, 2 * qq:2 * qq + 2, hsl],
                                    start=(qq == 0), stop=(qq == 1),
                                    perf_mode=DR,
                                )
                        v = ps.rearrange("p fb (h2 hr w2 wr) -> p fb h2 hr w2 wr",
                                         hr=2, w2=32, wr=2)
                        dst = g4[:, cg, 2 * fb2:2 * fb2 + 2]
                        nc.vector.tensor_copy(dst, v[:, :, :, 0, :, 0])
                        nc.vector.tensor_tensor(dst, dst, v[:, :, :, 0, :, 1], mx)
                        nc.vector.tensor_tensor(dst, dst, v[:, :, :, 1, :, 0], mx)
                        nc.vector.tensor_tensor(dst, dst, v[:, :, :, 1, :, 1], mx)

                # gT[m, c] via PE transpose of g[c, m] in 128x128 blocks
                for mc in range(MC):
                    pt = psT.tile([P, 2, P], bf16, tag="tr")
                    for cg in range(2):
                        nc.tensor.transpose(
                            pt[:, cg, :], g2[:, cg, mc * P:(mc + 1) * P], identity
                        )
                    nc.scalar.copy(gT_sb[:, mc, :], pt.rearrange("p a b -> p (a b)"))

            # ---------- attention ----------
            with (
                tc.tile_pool(name="psS", bufs=2, space="PSUM") as psS,
                tc.tile_pool(name="psO", bufs=1, space="PSUM") as psO,
                tc.tile_pool(name="psO2", bufs=2, space="PSUM") as psO2,
            ):
                pending = []

                def _emit_o2(item):
                    jnb, josb = item
                    jsl = slice(jnb * FB, (jnb + 1) * FB)
                    for ig in range(4):
                        o2 = psO2.tile([P, FB], fp32, tag="o2", name="o2")
                        for cg in range(2):
                            nc.tensor.matmul(
                                o2,
                                lhsT=wo2[:, cg, ig * P:(ig + 1) * P],
                                rhs=josb[:, cg, :],
                                start=(cg == 0), stop=(cg == 1),
                            )
                        ot = sb.tile([P, FB], fp16, tag="out", bufs=4, name="ot")
                        if ig < 2:
                            # DVE: fused residual add straight from PSUM
                            nc.vector.tensor_tensor(ot, o2, x16[:, ig, jsl], add)
                        else:
                            # ACT copy PSUM->SBUF, then GPSIMD (SBUF-only)
                            # residual add: balances the per-block DVE load
                            st = sb.tile([P, FB], fp32, tag="stage", bufs=4,
                                         name="st")
                            nc.scalar.copy(st, o2)
                            nc.gpsimd.tensor_tensor(ot, st, x16[:, ig, jsl], add)
                        nc.sync.dma_start(out_r[:, ig, jsl], ot)

                for nb in range(NB):
                    nsl = slice(nb * FB, (nb + 1) * FB)
                    # sT[m, n] = sum_c phi[c, m] * theta[c, n]: k=64, so two
                    # m-chunks run concurrently in disjoint PE row-halves
                    expT = sb.tile([P, MC, FB], bf16, tag="expT", bufs=2)
                    for mc2 in range(MC // 2):
                        ps = psS.tile([P, 2, FB], fp32, tag="sT")
                        nc.tensor.matmul(
                            ps[:, 0, :],
                            lhsT=phi_flat[:C8, (2 * mc2) * P:(2 * mc2 + 1) * P],
                            rhs=theta_sb[:C8, nsl],
                            start=True, stop=True, tile_position=(0, 0),
                        )
                        nc.tensor.matmul(
                            ps[:, 1, :],
                            lhsT=phi_flat[C8:, (2 * mc2 + 1) * P:(2 * mc2 + 2) * P],
                            rhs=theta_sb[C8:, nsl],
                            start=True, stop=True, tile_position=(64, 0),
                        )
                        nc.scalar.activation(
                            expT[:, 2 * mc2:2 * mc2 + 2, :].rearrange("p a b -> p (a b)"),
                            ps.rearrange("p a b -> p (a b)"), Exp,
                        )

                    # out-projection of the previous nb, emitted here so its
                    # PE matmuls and residual adds queue ahead of this nb's
                    # sums -> reciprocal -> scale chain
                    if pending:
                        _emit_o2(pending.pop(0))

                    # sum over m: 2-level DVE tree + GPSIMD tail reduces the
                    # 8 m-chunks to one [P, FB] tile, then ONE ones-matmul
                    # sums the 128 partitions (output rows all hold the sum,
                    # so 1/sum lands broadcast-ready)
                    part = sb.tile([P, 4, FB], bf16, tag="part", bufs=2)
                    nc.vector.tensor_tensor(
                        part.rearrange("p a b -> p (a b)"),
                        expT[:, 0:4, :].rearrange("p a b -> p (a b)"),
                        expT[:, 4:8, :].rearrange("p a b -> p (a b)"), add)
                    nc.vector.tensor_tensor(
                        part[:, 0:2, :].rearrange("p a b -> p (a b)"),
                        part[:, 0:2, :].rearrange("p a b -> p (a b)"),
                        part[:, 2:4, :].rearrange("p a b -> p (a b)"), add)
                    nc.gpsimd.tensor_tensor(
                        part[:, 0, :], part[:, 0, :], part[:, 1, :], add)
                    sum_ps = psS.tile([P, 2, FB], fp32, tag="sT",
                                      name="sum_ps")[:, 0, :]
                    nc.tensor.matmul(sum_ps, lhsT=ones_mat, rhs=part[:, 0, :],
                                     start=True, stop=True)
                    recipb = sb.tile([P, FB], fp32, tag="recipb", bufs=2)
                    nc.vector.reciprocal_approx_fast(recipb, sum_ps)

                    # o[c, n] = sum_m gT[m, c] * expT[m, n], normalized on the
                    # PSUM->SBUF copy by the per-column 1/sum
                    o_sb = sb.tile([P, 2, FB], bf16, tag="o_sb", bufs=2)
                    o_ps = psO.tile([P, 2, FB], fp32, tag="o_ps")
                    for cg in range(2):
                        for mc in range(MC):
                            nc.tensor.matmul(
                                o_ps[:, cg, :],
                                lhsT=gT_sb[:, mc, cg * P:(cg + 1) * P],
                                rhs=expT[:, mc, :],
                                start=(mc == 0), stop=(mc == MC - 1),
                            )
                    for cg in range(2):
                        nc.vector.tensor_tensor(o_sb[:, cg, :], o_ps[:, cg, :],
                                                recipb, mult)

                    pending.append((nb, o_sb))
                if pending:
                    _emit_o2(pending.pop(0))

    _strip_pe_self_waits(nc)
    nc.compile()
    return nc


def _get_nc():
    if "nc" not in _CACHE:
        _CACHE["nc"] = _build_nc()
    return _CACHE["nc"]


def make_in_maps(x, w_theta, w_phi, w_g, w_o, u_theta, u_phi, u_g, u_o, gamma):
    import ml_dtypes

    e4 = ml_dtypes.float8_e4m3
    wt = _sn(w_theta, u_theta).T                                  # [512, 64]
    wp = _sn(w_phi, u_phi).T                                      # [512, 64]
    wtp = np.ascontiguousarray(
        np.concatenate([wt, wp], axis=1).astype(np.float16))      # [512, 128]
    wg = np.ascontiguousarray(
        (WG_SCALE * _sn(w_g, u_g).T).astype(e4))                  # [512, 256]
    wo = np.ascontiguousarray(
        (np.float32(np.asarray(gamma, np.float32)) * _sn(w_o, u_o).T)
        .astype(ml_dtypes.bfloat16))                              # [256, 512]
    xf = np.asarray(x, np.float32).reshape(B, C, HW)
    x16 = xf.astype(np.float16)
    x8 = xf.astype(e4)
    return [
        {"x16": np.ascontiguousarray(x16[i]),
         "x8": np.ascontiguousarray(x8[i]),
         "wtp": wtp, "wg": wg, "wo": wo}
        for i in range(B)
    ]


def kernel(x, w_theta, w_phi, w_g, w_o, u_theta, u_phi, u_g, u_o, gamma):
    from concourse.bass_utils import run_bass_kernel_spmd

    in_maps = make_in_maps(
        x, w_theta, w_phi, w_g, w_o, u_theta, u_phi, u_g, u_o, gamma
    )
    nc = _get_nc()
    res = run_bass_kernel_spmd(nc, in_maps, core_ids=list(range(B)))
    out = np.stack([np.asarray(r["out"], np.float32) for r in res.results],
                   axis=0)
    return out.reshape(B, C, H, W)


# revision 3
# speedup vs baseline: 1.1029x; 1.1029x over previous
"""Self-attention (SAGAN-style, spectral-normalized 1x1 convs) on 8 TRN2 cores.

Contract: kernel(**inputs) takes the FULL unsharded inputs
(x [8,512,64,64], weights, power-iteration u vectors, gamma) and returns
the FULL output [8,512,64,64] (float32).

Sharding: data-parallel over batch B=8 -> one batch element per core.
Each core runs the complete attention block for its element; no
collectives are needed.

Per-core math (C=512, HW=4096, M=HW/4=1024):
    theta = sn(w_theta) @ x          [64, 4096]
    phi   = maxpool2(sn(w_phi) @ x)  [64, 1024]
    g     = maxpool2(sn(w_g)   @ x)  [256, 1024]
    sT[m,n] = sum_c phi[c,m] theta[c,n]
    beta  = softmax over m  (exp without max-subtraction: logits span
            ~+-51, safe in fp32/bf16; normalization applied to o)
    o     = (g @ exp(sT)) * (1/sum)  [256, 4096]
    out   = gamma * (sn(w_o) @ o) + x

Precision plan (measured 1.07e-2 output rel err vs the 2e-2 gate):
 - logit path fp16 (x16, w_theta|w_phi fused, theta, phi)
 - g-projection fp8 e4m3 DoubleRow (k=256/matmul); wg pre-scaled by 8 on
   the host so its entries are e4m3-normal, compensated exactly via the
   8.0-valued ones-matmul that makes the softmax denominators
 - value path: E=exp(sT) bf16, g/gT bf16, o stored e4m3, out-projection
   fp8 e4m3 DoubleRow with wo pre-scaled by 512*gamma; the 1/512 is
   folded into the residual ops (free scale slots), fp32 PSUM throughout
 - residual + output fp16 (host upcasts to fp32)

Schedule notes (from perfetto/ntff analysis of earlier versions):
 - FD=512 matmuls pipeline at ~216ns start-to-start incl. the implicit
   LDWEIGHTS, so wall time ~ matmul count; fp8 DoubleRow halves the
   count for k>=256 contractions (g-proj 32, out-proj 32/kernel).
 - softmax sums: 2-level DVE tree + TWO accumulating ones-matmuls
   replaces 8 matmuls/block; reciprocal_approx_fast replaces the 6.4
   cyc/elem exact reciprocal (27us -> 5.5us).
 - PE emission order per block interleaves the previous block's
   out-projection between sT pairs (fills the ACT exp-pipeline wait) and
   puts the sum-matmuls between the two o-matmul halves so the o-PSUM
   rotation never stalls; keeps PE dense so the HAM clock-gate stays at
   2.4GHz (idle gaps earlier cost ~25us of half-clock matmuls).
 - residual adds split DVE (scalar_tensor_tensor, ig 0-1) / ACT-copy +
   GPSIMD add (ig 2-3) to balance engine load; one out-DMA per block.
 - x arrives from the host pre-converted (fp16 + fp8) in 4 big DMAs per
   copy: zero on-device input conversion, ~10 input DMA dispatches.

PE->PE self-waits are stripped (PE->PSUM write port is FIFO) and bacc's
generate_event_semaphores legalizes the 1-wait ISA limit.

The spectral-norm power-iteration only involves [1,64]x[64,512]
matvecs, so it runs on the host in float32; gamma is folded into w_o.
"""

import numpy as np

B, C, H, W = 8, 512, 64, 64
HW = H * W            # 4096
M = HW // 4           # 1024 (pooled spatial)
C8 = C // 8           # 64
C2 = C // 2           # 256
P = 128               # SBUF partitions
KC = C // P           # 4 k-chunks for C-contraction
FB = 512              # free-dim block
NB = HW // FB         # 8 n-blocks
MC = M // P           # 8 m-chunks
WG_SCALE = 8.0        # host wg scale (e4m3 normal range), cancelled by
                      # using this value in the ones-matmul
WO_SCALE = 512.0      # host wo scale (e4m3 normal range), cancelled in
                      # the residual ops' scale slots
EPS = 1e-12

_CACHE = {}


def _sn(w, u):
    """Host-side spectral norm (eval-mode power iteration), float32."""
    w = np.asarray(w, np.float32)
    u = np.asarray(u, np.float32)
    v = u @ w
    v = v / max(np.float32(np.linalg.norm(v)), np.float32(EPS))
    u2 = v @ w.T
    u2 = u2 / max(np.float32(np.linalg.norm(u2)), np.float32(EPS))
    sv = np.float32((v @ w.T @ u2.T)[0, 0])
    return w / sv


def _strip_pe_self_waits(nc):
    """Remove S[PE]-waits from PE matmuls: PE->PE deps are ordered by the
    engine queue + FIFO PSUM write port, and matmuls only have one ISA
    wait slot."""
    import concourse.mybir as mybir

    for f in nc.m.functions:
        for blk in f.blocks:
            for inst in blk.instructions:
                if not isinstance(inst, mybir.InstMatmult):
                    continue
                si = inst.sync_info
                kept = [w for w in si.on_wait
                        if not (w.ant_name or "").startswith("PE_")]
                if len(kept) != len(si.on_wait):
                    si.on_wait = kept
                    inst.sync_info = si


def _build_nc():
    import concourse.bass as bass
    import concourse.mybir as mybir
    import concourse.tile as tile
    from concourse import bacc
    from concourse.masks import make_identity

    fp32 = mybir.dt.float32
    fp16 = mybir.dt.float16
    bf16 = mybir.dt.bfloat16
    fp8 = mybir.dt.float8e4
    DR = mybir.MatmulPerfMode.DoubleRow
    Exp = mybir.ActivationFunctionType.Exp
    mult = mybir.AluOpType.mult
    add = mybir.AluOpType.add
    mx = mybir.AluOpType.max

    nc = bacc.Bacc()
    x16_d = nc.dram_tensor("x16", [C, HW], fp16, kind="ExternalInput").ap()
    x8_d = nc.dram_tensor("x8", [C, HW], fp8, kind="ExternalInput").ap()
    wtp_d = nc.dram_tensor("wtp", [C, P], fp16, kind="ExternalInput").ap()
    wg_d = nc.dram_tensor("wg", [C, C2], fp8, kind="ExternalInput").ap()
    wo_d = nc.dram_tensor("wo", [C2, C], fp8, kind="ExternalInput").ap()
    out_d = nc.dram_tensor("out", [C, HW], fp16, kind="ExternalOutput").ap()

    x16_r = x16_d.rearrange("(kc p) n -> p kc n", p=P)
    x8_r = x8_d.rearrange("(kc p) n -> p kc n", p=P)
    out_r = out_d.rearrange("(ig p) n -> p ig n", p=P)

    with tile.TileContext(nc) as tc:
        with tc.tile_pool(name="sb", bufs=1) as sb:
            # ---- persistent tiles ----
            x16 = sb.tile([P, KC, HW], fp16)
            x8 = sb.tile([P, KC, HW], fp8)
            wtp = sb.tile([P, KC, P], fp16)
            wg8 = sb.tile([P, KC, C2], fp8)
            wo8 = sb.tile([P, 2, C], fp8)
            theta_sb = sb.tile([P, HW], fp16)             # rows 64:128 duplicate
            phi2 = sb.tile([P, NB, 4, 32], fp16)          # rows 64:128 duplicate
            g2 = sb.tile([P, 2, M], bf16)                 # pooled, = 8*g
            gT_sb = sb.tile([P, MC, C2], bf16)            # [m-part, mc, c]
            identity = sb.tile([P, P], bf16)
            ones_mat = sb.tile([P, P], bf16)              # value WG_SCALE

            phi_flat = phi2.rearrange("p a b c -> p (a b c)")
            g4 = g2.rearrange("p cg (fb h2 w2) -> p cg fb h2 w2", h2=4, w2=32)

            # ---- constants + HAM warmup ----
            nc.vector.memset(ones_mat, WG_SCALE)
            with tc.tile_pool(name="psW", bufs=1, space="PSUM") as psW:
                # junk matmuls keep the PE busy during the DMA head so the
                # HAM clock-gate opens before the real matmul stream
                wps = psW.tile([P, P], fp32, tag="warm")
                for _ in range(8):
                    nc.tensor.matmul(wps, lhsT=ones_mat, rhs=ones_mat,
                                     start=True, stop=True)
            ident_raw = sb.tile([P, P], fp32)
            make_identity(nc, ident_raw)
            nc.scalar.copy(identity, ident_raw)

            # ---- input DMAs: x16 first (theta/phi path), x8 trailing ----
            nc.sync.dma_start(wtp, wtp_d.rearrange("(kc p) i -> p kc i", p=P))
            nc.gpsimd.dma_start(wg8, wg_d.rearrange("(kc p) i -> p kc i", p=P))
            nc.sync.dma_start(wo8, wo_d.rearrange("(cg p) i -> p cg i", p=P))
            for fb2 in range(NB // 2):
                fsl = slice(2 * fb2 * FB, (2 * fb2 + 2) * FB)
                nc.sync.dma_start(x16[:, :, fsl], x16_r[:, :, fsl])
            for fb2 in range(NB // 2):
                fsl = slice(2 * fb2 * FB, (2 * fb2 + 2) * FB)
                nc.gpsimd.dma_start(x8[:, :, fsl], x8_r[:, :, fsl])

            # ---------- projections ----------
            with (
                tc.tile_pool(name="psA", bufs=3, space="PSUM") as psA,
                tc.tile_pool(name="psT", bufs=2, space="PSUM") as psT,
            ):
                # fused theta+phi projection: lhsT = [wt | wp] gives
                # theta on out-partitions 0:64, phi on 64:128; both results
                # duplicated onto partitions 64:128 for sT row-packing
                for fb2 in range(NB // 2):
                    ps = psA.tile([P, 2, FB], fp32, tag="proj", name="ps")
                    for half in range(2):
                        for kc in range(KC):
                            nc.tensor.matmul(
                                ps[:, half, :],
                                lhsT=wtp[:, kc, :],
                                rhs=x16[:, kc, (2 * fb2 + half) * FB:(2 * fb2 + half + 1) * FB],
                                start=(kc == 0), stop=(kc == KC - 1),
                            )
                    th = ps[:C8].rearrange("p a b -> p (a b)")
                    thsl = slice(2 * fb2 * FB, (2 * fb2 + 2) * FB)
                    nc.scalar.copy(theta_sb[:C8, thsl], th)
                    nc.vector.tensor_copy(theta_sb[C8:, thsl], theta_sb[:C8, thsl])
                    v = ps[C8:].rearrange("p fb (h2 hr w2 wr) -> p fb h2 hr w2 wr",
                                          hr=2, w2=32, wr=2)
                    dst = phi2[:C8, 2 * fb2:2 * fb2 + 2]   # [64, 2, 4, 32]
                    nc.vector.tensor_copy(dst, v[:, :, :, 0, :, 0])
                    nc.vector.tensor_tensor(dst, dst, v[:, :, :, 0, :, 1], mx)
                    nc.vector.tensor_tensor(dst, dst, v[:, :, :, 1, :, 0], mx)
                    nc.vector.tensor_tensor(dst, dst, v[:, :, :, 1, :, 1], mx)
                    nc.vector.tensor_copy(phi2[C8:, 2 * fb2:2 * fb2 + 2],
                                          phi2[:C8, 2 * fb2:2 * fb2 + 2])

                # g projection in fp8 DoubleRow (k=256/matmul) + maxpool,
                # then immediately transpose this fb2's two m-chunks
                for fb2 in range(NB // 2):
                    for cg in range(2):
                        ps = psA.tile([P, 2, FB], fp32, tag="proj", name="psg")
                        for half in range(2):
                            hsl = slice((2 * fb2 + half) * FB,
                                        (2 * fb2 + half + 1) * FB)
                            for qq in range(2):
                                nc.tensor.matmul(
                                    ps[:, half, :],
                                    lhsT=wg8[:, 2 * qq:2 * qq + 2, cg * P:(cg + 1) * P],
                                    rhs=x8[:, 2 * qq:2 * qq + 2, hsl],
                                    start=(qq == 0), stop=(qq == 1),
                                    perf_mode=DR,
                                )
                        v = ps.rearrange("p fb (h2 hr w2 wr) -> p fb h2 hr w2 wr",
                                         hr=2, w2=32, wr=2)
                        dst = g4[:, cg, 2 * fb2:2 * fb2 + 2]
                        nc.vector.tensor_copy(dst, v[:, :, :, 0, :, 0])
                        nc.vector.tensor_tensor(dst, dst, v[:, :, :, 0, :, 1], mx)
                        nc.vector.tensor_tensor(dst, dst, v[:, :, :, 1, :, 0], mx)
                        nc.vector.tensor_tensor(dst, dst, v[:, :, :, 1, :, 1], mx)
                    for mc in (2 * fb2, 2 * fb2 + 1):
                        pt = psT.tile([P, 2, P], bf16, tag="tr")
                        for cg in range(2):
                            nc.tensor.transpose(
                                pt[:, cg, :], g2[:, cg, mc * P:(mc + 1) * P],
                                identity,
                            )
                        nc.scalar.copy(gT_sb[:, mc, :],
                                       pt.rearrange("p a b -> p (a b)"))

            # ---------- attention ----------
            with (
                tc.tile_pool(name="psS", bufs=2, space="PSUM") as psS,
                tc.tile_pool(name="psO", bufs=2, space="PSUM") as psO,
                tc.tile_pool(name="psO2", bufs=2, space="PSUM") as psO2,
            ):
                pending = []

                def _emit_o2(item):
                    """Out-projection of block jnb: 4 fp8-DoubleRow matmuls
                    (k=256 each), residual adds split DVE / ACT+GPSIMD, one
                    out-DMA for all 512 output rows."""
                    jnb, jo8 = item
                    jsl = slice(jnb * FB, (jnb + 1) * FB)
                    ot4 = sb.tile([P, 4, FB], fp16, tag="out", bufs=2,
                                  name="ot4")
                    for ig in range(4):
                        o2 = psO2.tile([P, FB], fp32, tag="o2", name="o2")
                        nc.tensor.matmul(
                            o2,
                            lhsT=wo8[:, :, ig * P:(ig + 1) * P],
                            rhs=jo8,
                            start=True, stop=True, perf_mode=DR,
                        )
                        if ig < 2:
                            # DVE: (o2 * 1/WO_SCALE) + x16 in one op
                            nc.vector.scalar_tensor_tensor(
                                ot4[:, ig, :], o2, 1.0 / WO_SCALE,
                                x16[:, ig, jsl], op0=mult, op1=add)
                        else:
                            # ACT copy-with-scale PSUM->SBUF, GPSIMD add
                            st = sb.tile([P, FB], fp32, tag="stage", bufs=4,
                                         name="st")
                            nc.scalar.mul(st, o2, 1.0 / WO_SCALE)
                            nc.gpsimd.tensor_tensor(ot4[:, ig, :], st,
                                                    x16[:, ig, jsl], add)
                    nc.sync.dma_start(out_r[:, :, jsl], ot4)

                for nb in range(NB):
                    nsl = slice(nb * FB, (nb + 1) * FB)
                    # sT[m, n] = sum_c phi[c, m] theta[c, n]: k=64 pairs run
                    # concurrently in disjoint PE row-halves
                    expT = sb.tile([P, MC, FB], bf16, tag="expT", bufs=3)

                    def _st_pair(mc2):
                        ps = psS.tile([P, 2, FB], fp32, tag="sT")
                        nc.tensor.matmul(
                            ps[:, 0, :],
                            lhsT=phi_flat[:C8, (2 * mc2) * P:(2 * mc2 + 1) * P],
                            rhs=theta_sb[:C8, nsl],
                            start=True, stop=True, tile_position=(0, 0),
                        )
                        nc.tensor.matmul(
                            ps[:, 1, :],
                            lhsT=phi_flat[C8:, (2 * mc2 + 1) * P:(2 * mc2 + 2) * P],
                            rhs=theta_sb[C8:, nsl],
                            start=True, stop=True, tile_position=(64, 0),
                        )
                        nc.scalar.activation(
                            expT[:, 2 * mc2:2 * mc2 + 2, :].rearrange("p a b -> p (a b)"),
                            ps.rearrange("p a b -> p (a b)"), Exp,
                        )

                    _st_pair(0)
                    _st_pair(1)
                    # previous block's out-projection lands here: its PE
                    # matmuls fill the exp-pipeline wait of this block
                    if pending:
                        _emit_o2(pending.pop(0))
                    _st_pair(2)
                    _st_pair(3)

                    # sum over m: 2-level DVE tree, then two accumulating
                    # ones-matmuls sum the 128 partitions (output rows all
                    # hold the sum -> broadcast-ready for the DVE)
                    part = sb.tile([P, 4, FB], bf16, tag="part", bufs=2)
                    nc.vector.tensor_tensor(
                        part.rearrange("p a b -> p (a b)"),
                        expT[:, 0:4, :].rearrange("p a b -> p (a b)"),
                        expT[:, 4:8, :].rearrange("p a b -> p (a b)"), add)
                    nc.vector.tensor_tensor(
                        part[:, 0:2, :].rearrange("p a b -> p (a b)"),
                        part[:, 0:2, :].rearrange("p a b -> p (a b)"),
                        part[:, 2:4, :].rearrange("p a b -> p (a b)"), add)

                    # o[c, n] = sum_m gT[m, c] expT[m, n]; the sum-matmuls
                    # sit between the two cg halves so the recip is ready
                    # when the first half finishes accumulating
                    o8_sb = sb.tile([P, 2, FB], fp8, tag="o8", bufs=2)
                    o_ps0 = psO.tile([P, FB], fp32, tag="o_ps", name="o0")
                    for mc in range(MC):
                        nc.tensor.matmul(
                            o_ps0,
                            lhsT=gT_sb[:, mc, 0:P],
                            rhs=expT[:, mc, :],
                            start=(mc == 0), stop=(mc == MC - 1),
                        )
                    sum_ps = psS.tile([P, 2, FB], fp32, tag="sT",
                                      name="sum_ps")[:, 0, :]
                    nc.tensor.matmul(sum_ps, lhsT=ones_mat, rhs=part[:, 0, :],
                                     start=True, stop=False)
                    nc.tensor.matmul(sum_ps, lhsT=ones_mat, rhs=part[:, 1, :],
                                     start=False, stop=True)
                    recipb = sb.tile([P, FB], fp32, tag="recipb", bufs=2)
                    nc.vector.reciprocal_approx_fast(recipb, sum_ps)
                    o_ps1 = psO.tile([P, FB], fp32, tag="o_ps", name="o1")
                    for mc in range(MC):
                        nc.tensor.matmul(
                            o_ps1,
                            lhsT=gT_sb[:, mc, P:C2],
                            rhs=expT[:, mc, :],
                            start=(mc == 0), stop=(mc == MC - 1),
                        )
                    nc.vector.tensor_tensor(o8_sb[:, 0, :], o_ps0, recipb, mult)
                    nc.vector.tensor_tensor(o8_sb[:, 1, :], o_ps1, recipb, mult)

                    pending.append((nb, o8_sb))
                if pending:
                    _emit_o2(pending.pop(0))

    _strip_pe_self_waits(nc)
    nc.compile()
    return nc


def _get_nc():
    if "nc" not in _CACHE:
        _CACHE["nc"] = _build_nc()
    return _CACHE["nc"]


def make_in_maps(x, w_theta, w_phi, w_g, w_o, u_theta, u_phi, u_g, u_o, gamma):
    import ml_dtypes

    e4 = ml_dtypes.float8_e4m3
    wt = _sn(w_theta, u_theta).T                                  # [512, 64]
    wp = _sn(w_phi, u_phi).T                                      # [512, 64]
    wtp = np.ascontiguousarray(
        np.concatenate([wt, wp], axis=1).astype(np.float16))      # [512, 128]
    wg = np.ascontiguousarray(
        (WG_SCALE * _sn(w_g, u_g).T).astype(e4))                  # [512, 256]
    wo = np.ascontiguousarray(
        (WO_SCALE * np.float32(np.asarray(gamma, np.float32))
         * _sn(w_o, u_o).T).astype(e4))                           # [256, 512]
    xf = np.asarray(x, np.float32).reshape(B, C, HW)
    x16 = xf.astype(np.float16)
    x8 = xf.astype(e4)
    return [
        {"x16": np.ascontiguousarray(x16[i]),
         "x8": np.ascontiguousarray(x8[i]),
         "wtp": wtp, "wg": wg, "wo": wo}
        for i in range(B)
    ]


def kernel(x, w_theta, w_phi, w_g, w_o, u_theta, u_phi, u_g, u_o, gamma):
    from concourse.bass_utils import run_bass_kernel_spmd

    in_maps = make_in_maps(
        x, w_theta, w_phi, w_g, w_o, u_theta, u_phi, u_g, u_o, gamma
    )
    nc = _get_nc()
    res = run_bass_kernel_spmd(nc, in_maps, core_ids=list(range(B)))
    out = np.stack([np.asarray(r["out"], np.float32) for r in res.results],
                   axis=0)
    return out.reshape(B, C, H, W)


# revision 5
# speedup vs baseline: 1.2612x; 1.1435x over previous
"""Self-attention (SAGAN-style, spectral-normalized 1x1 convs) on 8 TRN2 cores.

Contract: kernel(**inputs) takes the FULL unsharded inputs
(x [8,512,64,64], weights, power-iteration u vectors, gamma) and returns
the FULL output [8,512,64,64] (float32).

Sharding: data-parallel over batch B=8 -> one batch element per core.
Each core runs the complete attention block for its element; no
collectives are needed.

Per-core math (C=512, HW=4096, M=HW/4=1024):
    theta = sn(w_theta) @ x          [64, 4096]
    phi   = maxpool2(sn(w_phi) @ x)  [64, 1024]
    g     = maxpool2(sn(w_g)   @ x)  [256, 1024]
    sT[m,n] = sum_c phi[c,m] theta[c,n]
    beta  = softmax over m  (exp without max-subtraction: logits span
            ~+-51, safe in fp32/bf16; normalization applied to o)
    o     = (g @ exp(sT)) * (1/sum)  [256, 4096]
    out   = gamma * (sn(w_o) @ o) + x

Precision plan (measured 9.4e-3 output rel err vs the 2e-2 gate):
 - projections + logits fp16 (x16 from host, theta, phi, wg), bf16 E/g
 - o stored e4m3; out-projection fp8 e4m3 DoubleRow (k=256 per matmul)
   with wo pre-scaled by 512*gamma on the host; the 1/512 rides in the
   scale slot of the residual ops; fp32 PSUM throughout
 - residual + output fp16 (host upcasts to fp32)

Schedule notes (from perfetto/ntff analysis of earlier versions):
 - FD=512 matmuls pipeline at ~216ns start-to-start (incl. implicit
   LDWEIGHTS), so wall time ~ matmul slot count. Counts per core:
   theta/phi 32, g 64, transposes 16, per block: 4 packed sT pairs,
   2 accumulating ones-matmuls, 16 o, 4 fp8-DR out-proj.
 - input = ONE 4MB fp16 x copy (the fp8 x copy of earlier versions was
   dropped: input DMA bandwidth ~230GB/s paces the head, PE has slack
   there, so fp16 g-matmuls are free and more accurate); x chunks
   alternate sync/vector DMA queues, weights ride gpsimd.
 - ~14 junk warmup matmuls bridge the DMA head so the HAM clock-gate
   never re-throttles the PE to 1.2GHz (cost ~25us in earlier runs).
 - softmax sums: 2-level DVE tree + two accumulating ones-matmuls
   (replaces 8 matmuls/block); reciprocal_approx_fast (not the 6.4
   cyc/elem exact reciprocal); sums emitted between the two o-matmul
   halves so the o-PSUM rotation never stalls.
 - out-projection of block nb-1 is emitted split: PE matmuls + 2 DVE
   fused residuals (scalar_tensor_tensor) early (fills the exp-pipeline
   wait), ACT copy-scale + GPSIMD adds + single out-DMA after the sT
   pairs so ACT's exp stream is never interrupted.

PE->PE self-waits are stripped (PE->PSUM write port is FIFO) and bacc's
generate_event_semaphores legalizes the 1-wait ISA limit.

The spectral-norm power-iteration only involves [1,64]x[64,512]
matvecs, so it runs on the host in float32; gamma is folded into w_o.
"""

import numpy as np

B, C, H, W = 8, 512, 64, 64
HW = H * W            # 4096
M = HW // 4           # 1024 (pooled spatial)
C8 = C // 8           # 64
C2 = C // 2           # 256
P = 128               # SBUF partitions
KC = C // P           # 4 k-chunks for C-contraction
FB = 512              # free-dim block
NB = HW // FB         # 8 n-blocks
MC = M // P           # 8 m-chunks
WO_SCALE = 512.0      # host wo scale (e4m3 normal range), cancelled in
                      # the residual ops' scale slots
EPS = 1e-12

_CACHE = {}


def _sn(w, u):
    """Host-side spectral norm (eval-mode power iteration), float32."""
    w = np.asarray(w, np.float32)
    u = np.asarray(u, np.float32)
    v = u @ w
    v = v / max(np.float32(np.linalg.norm(v)), np.float32(EPS))
    u2 = v @ w.T
    u2 = u2 / max(np.float32(np.linalg.norm(u2)), np.float32(EPS))
    sv = np.float32((v @ w.T @ u2.T)[0, 0])
    return w / sv


def _strip_pe_self_waits(nc):
    """Remove S[PE]-waits from PE matmuls: PE->PE deps are ordered by the
    engine queue + FIFO PSUM write port, and matmuls only have one ISA
    wait slot."""
    import concourse.mybir as mybir

    for f in nc.m.functions:
        for blk in f.blocks:
            for inst in blk.instructions:
                if not isinstance(inst, mybir.InstMatmult):
                    continue
                si = inst.sync_info
                kept = [w for w in si.on_wait
                        if not (w.ant_name or "").startswith("PE_")]
                if len(kept) != len(si.on_wait):
                    si.on_wait = kept
                    inst.sync_info = si


def _build_nc():
    import concourse.bass as bass
    import concourse.mybir as mybir
    import concourse.tile as tile
    from concourse import bacc
    from concourse.masks import make_identity

    fp32 = mybir.dt.float32
    fp16 = mybir.dt.float16
    bf16 = mybir.dt.bfloat16
    fp8 = mybir.dt.float8e4
    DR = mybir.MatmulPerfMode.DoubleRow
    Exp = mybir.ActivationFunctionType.Exp
    mult = mybir.AluOpType.mult
    add = mybir.AluOpType.add
    mx = mybir.AluOpType.max

    nc = bacc.Bacc()
    x16_d = nc.dram_tensor("x16", [C, HW], fp16, kind="ExternalInput").ap()
    wtp_d = nc.dram_tensor("wtp", [C, P], fp16, kind="ExternalInput").ap()
    wg_d = nc.dram_tensor("wg", [C, C2], fp16, kind="ExternalInput").ap()
    wo_d = nc.dram_tensor("wo", [C2, C], fp8, kind="ExternalInput").ap()
    out_d = nc.dram_tensor("out", [C, HW], fp16, kind="ExternalOutput").ap()

    x16_r = x16_d.rearrange("(kc p) n -> p kc n", p=P)
    out_r = out_d.rearrange("(ig p) n -> p ig n", p=P)

    with tile.TileContext(nc) as tc:
        with tc.tile_pool(name="sb", bufs=1) as sb:
            # ---- persistent tiles ----
            x16 = sb.tile([P, KC, HW], fp16)
            wtp = sb.tile([P, KC, P], fp16)
            wg2 = sb.tile([P, KC, C2], fp16)
            wo8 = sb.tile([P, 2, C], fp8)
            theta_sb = sb.tile([P, HW], fp16)             # rows 64:128 duplicate
            phi2 = sb.tile([P, NB, 4, 32], fp16)          # rows 64:128 duplicate
            g2 = sb.tile([P, 2, M], bf16)                 # pooled, cg-major
            gT_sb = sb.tile([P, MC, C2], bf16)            # [m-part, mc, c]
            identity = sb.tile([P, P], bf16)
            ones_mat = sb.tile([P, P], bf16)

            phi_flat = phi2.rearrange("p a b c -> p (a b c)")
            g4 = g2.rearrange("p cg (fb h2 w2) -> p cg fb h2 w2", h2=4, w2=32)

            # ---- constants + HAM warmup ----
            nc.vector.memset(ones_mat, 1.0)
            with tc.tile_pool(name="psW", bufs=1, space="PSUM") as psW:
                # junk matmuls keep the PE busy during the DMA head so the
                # HAM clock-gate opens before the real matmul stream
                wps = psW.tile([P, P], fp32, tag="warm")
                for _ in range(14):
                    nc.tensor.matmul(wps, lhsT=ones_mat, rhs=ones_mat,
                                     start=True, stop=True)
            ident_raw = sb.tile([P, P], fp32)
            make_identity(nc, ident_raw)
            nc.scalar.copy(identity, ident_raw)

            # ---- input DMAs: weights first, x chunks on two queues ----
            nc.sync.dma_start(wtp, wtp_d.rearrange("(kc p) i -> p kc i", p=P))
            nc.gpsimd.dma_start(wg2, wg_d.rearrange("(kc p) i -> p kc i", p=P))
            nc.gpsimd.dma_start(wo8, wo_d.rearrange("(cg p) i -> p cg i", p=P))
            for q in range(NB):
                sl = slice(q * FB, (q + 1) * FB)
                eng = nc.sync if q % 2 == 0 else nc.scalar
                eng.dma_start(x16[:, :, sl], x16_r[:, :, sl])

            # ---------- projections ----------
            with (
                tc.tile_pool(name="psA", bufs=3, space="PSUM") as psA,
                tc.tile_pool(name="psT", bufs=2, space="PSUM") as psT,
            ):
                for fb2 in range(NB // 2):
                    # fused theta+phi projection: lhsT = [wt | wp] gives
                    # theta on out-partitions 0:64, phi on 64:128; both
                    # duplicated onto partitions 64:128 for sT row-packing
                    ps = psA.tile([P, 2, FB], fp32, tag="proj", name="ps")
                    for half in range(2):
                        for kc in range(KC):
                            nc.tensor.matmul(
                                ps[:, half, :],
                                lhsT=wtp[:, kc, :],
                                rhs=x16[:, kc, (2 * fb2 + half) * FB:(2 * fb2 + half + 1) * FB],
                                start=(kc == 0), stop=(kc == KC - 1),
                            )
                    th = ps[:C8].rearrange("p a b -> p (a b)")
                    thsl = slice(2 * fb2 * FB, (2 * fb2 + 2) * FB)
                    nc.scalar.copy(theta_sb[:C8, thsl], th)
                    nc.vector.tensor_copy(theta_sb[C8:, thsl], theta_sb[:C8, thsl])
                    v = ps[C8:].rearrange("p fb (h2 hr w2 wr) -> p fb h2 hr w2 wr",
                                          hr=2, w2=32, wr=2)
                    dst = phi2[:C8, 2 * fb2:2 * fb2 + 2]   # [64, 2, 4, 32]
                    nc.vector.tensor_copy(dst, v[:, :, :, 0, :, 0])
                    nc.vector.tensor_tensor(dst, dst, v[:, :, :, 0, :, 1], mx)
                    nc.vector.tensor_tensor(dst, dst, v[:, :, :, 1, :, 0], mx)
                    nc.vector.tensor_tensor(dst, dst, v[:, :, :, 1, :, 1], mx)
                    nc.vector.tensor_copy(phi2[C8:, 2 * fb2:2 * fb2 + 2],
                                          phi2[:C8, 2 * fb2:2 * fb2 + 2])

                    # g projection + maxpool on the same x16 columns, then
                    # transpose this fb2's two m-chunks while they're hot
                    for cg in range(2):
                        ps = psA.tile([P, 2, FB], fp32, tag="proj", name="psg")
                        for half in range(2):
                            for kc in range(KC):
                                nc.tensor.matmul(
                                    ps[:, half, :],
                                    lhsT=wg2[:, kc, cg * P:(cg + 1) * P],
                                    rhs=x16[:, kc, (2 * fb2 + half) * FB:(2 * fb2 + half + 1) * FB],
                                    start=(kc == 0), stop=(kc == KC - 1),
                                )
                        v = ps.rearrange("p fb (h2 hr w2 wr) -> p fb h2 hr w2 wr",
                                         hr=2, w2=32, wr=2)
                        dst = g4[:, cg, 2 * fb2:2 * fb2 + 2]
                        nc.vector.tensor_copy(dst, v[:, :, :, 0, :, 0])
                        nc.vector.tensor_tensor(dst, dst, v[:, :, :, 0, :, 1], mx)
                        nc.vector.tensor_tensor(dst, dst, v[:, :, :, 1, :, 0], mx)
                        nc.vector.tensor_tensor(dst, dst, v[:, :, :, 1, :, 1], mx)
                    for mc in (2 * fb2, 2 * fb2 + 1):
                        pt = psT.tile([P, 2, P], bf16, tag="tr")
                        for cg in range(2):
                            nc.tensor.transpose(
                                pt[:, cg, :], g2[:, cg, mc * P:(mc + 1) * P],
                                identity,
                            )
                        nc.scalar.copy(gT_sb[:, mc, :],
                                       pt.rearrange("p a b -> p (a b)"))

            # ---------- attention ----------
            with (
                tc.tile_pool(name="psS", bufs=2, space="PSUM") as psS,
                tc.tile_pool(name="psO", bufs=2, space="PSUM") as psO,
                tc.tile_pool(name="psO2", bufs=2, space="PSUM") as psO2,
            ):
                pending = []

                def _emit_o2_early(item):
                    """Out-projection matmuls of block jnb (4 fp8-DR, k=256)
                    + the 2 DVE fused residuals. PE work lands early to fill
                    the exp-pipeline wait of the current block."""
                    jnb, jo8 = item
                    jsl = slice(jnb * FB, (jnb + 1) * FB)
                    ot4 = sb.tile([P, 4, FB], fp16, tag="out", bufs=2,
                                  name="ot4")
                    o2s = []
                    for ig in range(4):
                        o2 = psO2.tile([P, FB], fp32, tag="o2", name="o2")
                        nc.tensor.matmul(
                            o2,
                            lhsT=wo8[:, :, ig * P:(ig + 1) * P],
                            rhs=jo8,
                            start=True, stop=True, perf_mode=DR,
                        )
                        if ig < 2:
                            # DVE: (o2 * 1/WO_SCALE) + x16 in one op
                            nc.vector.scalar_tensor_tensor(
                                ot4[:, ig, :], o2, 1.0 / WO_SCALE,
                                x16[:, ig, jsl], op0=mult, op1=add)
                        else:
                            o2s.append(o2)
                    return (jnb, ot4, o2s)

                def _emit_o2_late(item):
                    """ACT copy-with-scale + GPSIMD residual adds for ig 2,3
                    and the single out-DMA; emitted after the sT pairs so
                    ACT's exp stream is not interrupted."""
                    jnb, ot4, o2s = item
                    jsl = slice(jnb * FB, (jnb + 1) * FB)
                    for ig, o2 in zip((2, 3), o2s):
                        st = sb.tile([P, FB], fp32, tag="stage", bufs=4,
                                     name="st")
                        nc.scalar.mul(st, o2, 1.0 / WO_SCALE)
                        nc.gpsimd.tensor_tensor(ot4[:, ig, :], st,
                                                x16[:, ig, jsl], add)
                    nc.sync.dma_start(out_r[:, :, jsl], ot4)

                for nb in range(NB):
                    nsl = slice(nb * FB, (nb + 1) * FB)
                    # sT[m, n] = sum_c phi[c, m] theta[c, n]: k=64 pairs run
                    # concurrently in disjoint PE row-halves
                    expT = sb.tile([P, MC, FB], bf16, tag="expT", bufs=3)

                    def _st_pair(mc2):
                        ps = psS.tile([P, 2, FB], fp32, tag="sT")
                        nc.tensor.matmul(
                            ps[:, 0, :],
                            lhsT=phi_flat[:C8, (2 * mc2) * P:(2 * mc2 + 1) * P],
                            rhs=theta_sb[:C8, nsl],
                            start=True, stop=True, tile_position=(0, 0),
                        )
                        nc.tensor.matmul(
                            ps[:, 1, :],
                            lhsT=phi_flat[C8:, (2 * mc2 + 1) * P:(2 * mc2 + 2) * P],
                            rhs=theta_sb[C8:, nsl],
                            start=True, stop=True, tile_position=(64, 0),
                        )
                        nc.scalar.activation(
                            expT[:, 2 * mc2:2 * mc2 + 2, :].rearrange("p a b -> p (a b)"),
                            ps.rearrange("p a b -> p (a b)"), Exp,
                        )

                    _st_pair(0)
                    _st_pair(1)
                    # previous block's out-proj matmuls fill the exp wait
                    held = _emit_o2_early(pending.pop(0)) if pending else None
                    _st_pair(2)
                    _st_pair(3)
                    if held is not None:
                        _emit_o2_late(held)

                    # sum over m: 2-level DVE tree, then two accumulating
                    # ones-matmuls sum the 128 partitions (output rows all
                    # hold the sum -> broadcast-ready for the DVE)
                    part = sb.tile([P, 4, FB], bf16, tag="part", bufs=2)
                    nc.vector.tensor_tensor(
                        part.rearrange("p a b -> p (a b)"),
                        expT[:, 0:4, :].rearrange("p a b -> p (a b)"),
                        expT[:, 4:8, :].rearrange("p a b -> p (a b)"), add)
                    nc.vector.tensor_tensor(
                        part[:, 0:2, :].rearrange("p a b -> p (a b)"),
                        part[:, 0:2, :].rearrange("p a b -> p (a b)"),
                        part[:, 2:4, :].rearrange("p a b -> p (a b)"), add)

                    # o[c, n] = sum_m gT[m, c] expT[m, n]; sums sit between
                    # the two cg halves so the reciprocal is ready when the
                    # first half finishes accumulating
                    o8_sb = sb.tile([P, 2, FB], fp8, tag="o8", bufs=2)
                    o_ps0 = psO.tile([P, FB], fp32, tag="o_ps", name="o0")
                    for mc in range(MC):
                        nc.tensor.matmul(
                            o_ps0,
                            lhsT=gT_sb[:, mc, 0:P],
                            rhs=expT[:, mc, :],
                            start=(mc == 0), stop=(mc == MC - 1),
                        )
                    sum_ps = psS.tile([P, 2, FB], fp32, tag="sT",
                                      name="sum_ps")[:, 0, :]
                    nc.tensor.matmul(sum_ps, lhsT=ones_mat, rhs=part[:, 0, :],
                                     start=True, stop=False)
                    nc.tensor.matmul(sum_ps, lhsT=ones_mat, rhs=part[:, 1, :],
                                     start=False, stop=True)
                    recipb = sb.tile([P, FB], fp32, tag="recipb", bufs=2)
                    nc.vector.reciprocal_approx_fast(recipb, sum_ps)
                    o_ps1 = psO.tile([P, FB], fp32, tag="o_ps", name="o1")
                    for mc in range(MC):
                        nc.tensor.matmul(
                            o_ps1,
                            lhsT=gT_sb[:, mc, P:C2],
                            rhs=expT[:, mc, :],
                            start=(mc == 0), stop=(mc == MC - 1),
                        )
                    nc.vector.tensor_tensor(o8_sb[:, 0, :], o_ps0, recipb, mult)
                    nc.vector.tensor_tensor(o8_sb[:, 1, :], o_ps1, recipb, mult)

                    pending.append((nb, o8_sb))
                if pending:
                    _emit_o2_late(_emit_o2_early(pending.pop(0)))

    _strip_pe_self_waits(nc)
    nc.compile()
    return nc


def _get_nc():
    if "nc" not in _CACHE:
        _CACHE["nc"] = _build_nc()
    return _CACHE["nc"]


def make_in_maps(x, w_theta, w_phi, w_g, w_o, u_theta, u_phi, u_g, u_o, gamma):
    import ml_dtypes

    e4 = ml_dtypes.float8_e4m3
    wt = _sn(w_theta, u_theta).T                                  # [512, 64]
    wp = _sn(w_phi, u_phi).T                                      # [512, 64]
    wtp = np.ascontiguousarray(
        np.concatenate([wt, wp], axis=1).astype(np.float16))      # [512, 128]
    wg = np.ascontiguousarray(_sn(w_g, u_g).T.astype(np.float16)) # [512, 256]
    wo = np.ascontiguousarray(
        (WO_SCALE * np.float32(np.asarray(gamma, np.float32))
         * _sn(w_o, u_o).T).astype(e4))                           # [256, 512]
    xf = np.asarray(x, np.float32).reshape(B, C, HW)
    x16 = xf.astype(np.float16)
    return [
        {"x16": np.ascontiguousarray(x16[i]),
         "wtp": wtp, "wg": wg, "wo": wo}
        for i in range(B)
    ]


def kernel(x, w_theta, w_phi, w_g, w_o, u_theta, u_phi, u_g, u_o, gamma):
    from concourse.bass_utils import run_bass_kernel_spmd

    in_maps = make_in_maps(
        x, w_theta, w_phi, w_g, w_o, u_theta, u_phi, u_g, u_o, gamma
    )
    nc = _get_nc()
    res = run_bass_kernel_spmd(nc, in_maps, core_ids=list(range(B)))
    out = np.stack([np.asarray(r["out"], np.float32) for r in res.results],
                   axis=0)
    return out.reshape(B, C, H, W)
